# revision 1
# baseline (speedup 1.0000x reference)
"""Multi-head differential attention on 8 Trainium2 NeuronCores.

Sharding: data-parallel over batch (B=2) x tensor-parallel over heads
(16 heads -> 4 per core). Core c handles batch c//4 and heads
4*(c%4) .. 4*(c%4)+3. Each core computes its heads' attention output and a
partial output projection; the host sums the 4 partials per batch.

All heavy matmuls run in float32r (full PE rate) with fp32 storage.
"""

import math
import os
import sys

sys.path.insert(0, "/opt/trn_rl_repo")

import numpy as np

B, S, HID, NH = 2, 2048, 2048, 16
HD = HID // NH          # 128
QKD = HD // 2           # 64
NCORES = 8
GRPS = NCORES // B      # head groups per batch
HPC = NH // GRPS        # heads per core = 4
LAYER_ID = 1
LAMBDA_INIT = 0.8 - 0.6 * math.exp(-0.3 * LAYER_ID)
EPS = 1e-6

NB = S // 512           # 4 seq blocks of 512
NKC = S // 128          # 16 key chunks of 128

_PROGRAM = None         # compiled bass program, reused across calls


def _build_program():
    import concourse.bass as bass
    import concourse.tile as tile
    from concourse import bacc, mybir

    f32 = mybir.dt.float32
    f32r = mybir.dt.float32r
    bf16 = mybir.dt.bfloat16
    Alu = mybir.AluOpType
    Act = mybir.ActivationFunctionType

    nc = bacc.Bacc(None, target_bir_lowering=False, debug=False)

    def din(name, shape, dt=f32):
        return nc.dram_tensor(name, shape, dt, kind="ExternalInput").ap()

    io = {
        "xq_t": din("xq_t", [HID, S], f32r),
        "xk_t": din("xk_t", [HID, S], f32r),
        "xv_t": din("xv_t", [HID, S], f32r),
        "wq_t": din("wq_t", [HID, 512], f32r),
        "wk_t": din("wk_t", [HID, 512], f32r),
        "wv_t": din("wv_t", [HID, 512], f32r),
        "wo_t": din("wo_t", [512, HID], f32r),
        "crep": din("crep", [128, S]),
        "srep": din("srep", [128, S]),
        "pmat": din("pmat", [128, 128], f32r),
        "negi": din("negi", [128, 128], bf16),
        "utri": din("utri", [128, 128], bf16),
        "ones_a": din("ones_a", [128, 128], f32r),
        "neglam": din("neglam", [128, 1]),
    }
    y_t = nc.dram_tensor("y_t", [HID, S], f32, kind="ExternalOutput").ap()

    from contextlib import ExitStack

    with tile.TileContext(nc) as tc, ExitStack() as ctx:
        persist = ctx.enter_context(tc.tile_pool(name="persist", bufs=1))
        constp = ctx.enter_context(tc.tile_pool(name="constp", bufs=1))

        # constants
        crep = constp.tile([128, S], f32, name="crep_sb", tag="crep")
        srep = constp.tile([128, S], f32, name="srep_sb", tag="srep")
        pmat = constp.tile([128, 128], f32r, name="pmat_sb", tag="pmat")
        negi = constp.tile([128, 128], bf16, name="negi_sb", tag="negi")
        utri = constp.tile([128, 128], bf16, name="utri_sb", tag="utri")
        ones_a = constp.tile([128, 128], f32r, name="ones_a_sb", tag="ones_a")
        neglam = constp.tile([128, 1], f32, name="neglam_sb", tag="neglam")
        epsb = constp.tile([128, 1], f32, name="epsb", tag="epsb")
        nc.vector.memset(epsb[:], EPS)
        for t, key in ((crep, "crep"), (srep, "srep"), (pmat, "pmat"),
                       (negi, "negi"), (utri, "utri"), (ones_a, "ones_a"),
                       (neglam, "neglam")):
            nc.sync.dma_start(out=t[:], in_=io[key][:])

        # persistent tensors: Q^T/K^T per (map g, head-pair hp): [128, S]
        #   tile t = 2*g + hp; partitions [64*a, 64*a+64) hold head 2*hp+a.
        QT = [persist.tile([128, S], f32r, name=f"qt{t}", tag=f"qt{t}")
              for t in range(4)]
        KT = [persist.tile([128, S], f32r, name=f"kt{t}", tag=f"kt{t}")
              for t in range(4)]
        # V natural layout per 128-seq chunk: [128 seq, 4 heads * 128 feat]
        VH = [persist.tile([128, 512], f32r, name=f"vh{s}", tag=f"vh{s}")
              for s in range(NKC)]
        # combined attention output (pre/post RMS), transposed: [feat, seq]
        U = [persist.tile([128, S], f32r, name=f"u{h}", tag=f"u{h}")
             for h in range(HPC)]

        # ---------------- phase P: q/k/v projections ----------------
        with tc.tile_pool(name="wp", bufs=1) as wp, \
             tc.tile_pool(name="xp", bufs=4) as xp, \
             tc.tile_pool(name="pp", bufs=1, space="PSUM") as pp:
            for wname, xname, mode in (("wq_t", "xq_t", 0), ("wk_t", "xk_t", 1),
                                       ("wv_t", "xv_t", 2)):
                wt = [wp.tile([128, 512], f32r, name=f"{wname}_{kc}", tag=f"w{kc}")
                      for kc in range(NKC)]
                for kc in range(NKC):
                    nc.sync.dma_start(out=wt[kc][:],
                                      in_=io[wname][kc * 128:(kc + 1) * 128, :])
                xin = io[xname]
                for n in range(NB):
                    ps = [pp.tile([128, 512], f32, name=f"pp{t}_{mode}_{n}",
                                  tag=f"pp{t}") for t in range(4)]
                    for kc in range(NKC):
                        xck = xp.tile([128, 512], f32r, name=f"x_{mode}_{n}_{kc}",
                                      tag="x")
                        nc.sync.dma_start(
                            out=xck[:],
                            in_=xin[kc * 128:(kc + 1) * 128,
                                    n * 512:(n + 1) * 512])
                        for t in range(4):
                            if mode < 2:
                                lhsT = wt[kc][:, t * 128:(t + 1) * 128]
                                rhs = xck[:]
                            else:
                                lhsT = xck[:, t * 128:(t + 1) * 128]
                                rhs = wt[kc][:]
                            nc.tensor.matmul(ps[t][:], lhsT, rhs,
                                             start=(kc == 0), stop=(kc == NKC - 1))
                    for t in range(4):
                        if mode == 0:
                            nc.vector.tensor_copy(QT[t][:, n * 512:(n + 1) * 512],
                                                  ps[t][:])
                        elif mode == 1:
                            nc.vector.tensor_copy(KT[t][:, n * 512:(n + 1) * 512],
                                                  ps[t][:])
                        else:
                            nc.vector.tensor_copy(VH[n * 4 + t][:], ps[t][:])

        # ---------------- phase R: rope on Q^T and K^T ----------------
        # y = x*cos + (P x)*sin, P = fixed signed pair-swap (via PE matmul)
        with tc.tile_pool(name="rp", bufs=2, space="PSUM") as rp, \
             tc.tile_pool(name="rt", bufs=2) as rtp:
            for T in (QT, KT):
                for t in range(4):
                    px = rp.tile([128, S], f32, name=f"px_{T[t].name}", tag="px")
                    for n in range(NB):
                        nc.tensor.matmul(px[:, n * 512:(n + 1) * 512],
                                         pmat[:],
                                         T[t][:, n * 512:(n + 1) * 512],
                                         start=True, stop=True)
                    tmp = rtp.tile([128, S], f32, name=f"rtmp_{T[t].name}",
                                   tag="rtmp")
                    nc.vector.tensor_mul(tmp[:], px[:], srep[:])
                    nc.vector.tensor_mul(T[t][:],
                                         T[t][:].bitcast(f32), crep[:])
                    nc.vector.tensor_add(T[t][:], T[t][:].bitcast(f32), tmp[:])

        # ---------------- phase A: attention ----------------
        # scores^T per 128-key chunk; psum tile [128, 1024] = both maps
        with tc.tile_pool(name="sp", bufs=2, space="PSUM") as sp, \
             tc.tile_pool(name="pvp", bufs=1, space="PSUM") as pvp, \
             tc.tile_pool(name="smp", bufs=1, space="PSUM") as smp, \
             tc.tile_pool(name="ep", bufs=3) as ep, \
             tc.tile_pool(name="cb", bufs=2) as cb:
            for h in range(HPC):
                hp, a = h // 2, h % 2
                poff = 64 * a
                for qb in range(NB):
                    pv1 = pvp.tile([128, 512], f32, name=f"pv1_{h}_{qb}", tag="pv1")
                    pv2 = pvp.tile([128, 512], f32, name=f"pv2_{h}_{qb}", tag="pv2")
                    sm = smp.tile([128, 1024], f32, name=f"sm_{h}_{qb}", tag="sm")
                    nkc = 4 * qb + 4
                    for kc in range(nkc):
                        j = kc - 4 * qb  # >= 0 on the causal diagonal band
                        qoff = j * 128 if j >= 0 else 0
                        ps = sp.tile([128, 1024], f32, name=f"s_{h}_{qb}_{kc}",
                                     tag="s")
                        for g in (0, 1):
                            tq = 2 * g + hp
                            nc.tensor.matmul(
                                ps[:, g * 512 + qoff:g * 512 + 512],
                                KT[tq][poff:poff + 64,
                                       kc * 128:(kc + 1) * 128],
                                QT[tq][poff:poff + 64,
                                       qb * 512 + qoff:(qb + 1) * 512],
                                start=True, stop=(j < 0))
                            if j >= 0:
                                nc.tensor.matmul(
                                    ps[:, g * 512 + qoff:g * 512 + qoff + 128],
                                    negi[:], utri[:], start=False, stop=True)
                        E = ep.tile([128, 1024], f32r, name=f"e_{h}_{qb}_{kc}",
                                    tag="e")
                        if qoff == 0:
                            nc.scalar.activation(E[:], ps[:], Act.Exp, scale=0.125)
                        else:
                            for g in (0, 1):
                                nc.scalar.activation(
                                    E[:, g * 512 + qoff:g * 512 + 512],
                                    ps[:, g * 512 + qoff:g * 512 + 512],
                                    Act.Exp, scale=0.125)
                        first, last = (kc == 0), (kc == nkc - 1)
                        for g in (0, 1):
                            pv = pv1 if g == 0 else pv2
                            nc.tensor.matmul(
                                pv[:, qoff:512],
                                VH[kc][:, h * 128:(h + 1) * 128],
                                E[:, g * 512 + qoff:g * 512 + 512],
                                start=first, stop=last)
                            nc.tensor.matmul(
                                sm[:, g * 512 + qoff:g * 512 + 512],
                                ones_a[:],
                                E[:, g * 512 + qoff:g * 512 + 512],
                                start=first, stop=last)
                    # combine: U = pv1/sm1 - lam * pv2/sm2
                    rb = cb.tile([128, 1024], f32, name=f"rb_{h}_{qb}", tag="rb")
                    nc.vector.reciprocal_approx_fast(rb[:], sm[:])
                    t1 = cb.tile([128, 512], f32, name=f"t1_{h}_{qb}", tag="t1")
                    t2 = cb.tile([128, 512], f32, name=f"t2_{h}_{qb}", tag="t2")
                    nc.vector.tensor_mul(t1[:], pv1[:], rb[:, 0:512])
                    nc.vector.tensor_mul(t2[:], pv2[:], rb[:, 512:1024])
                    # U = (t2 * -lam) + t1
                    nc.vector.scalar_tensor_tensor(
                        U[h][:, qb * 512:(qb + 1) * 512], t2[:], neglam[:], t1[:],
                        op0=Alu.mult, op1=Alu.add)

        # ---------------- phase N: RMS norm over feat (partition) dim ----------
        with tc.tile_pool(name="np_ps", bufs=2, space="PSUM") as npp, \
             tc.tile_pool(name="np_sb", bufs=2) as nsb:
            for h in range(HPC):
                for qb in range(NB):
                    usl = U[h][:, qb * 512:(qb + 1) * 512]
                    sq = nsb.tile([128, 512], f32r, name=f"sq_{h}_{qb}", tag="sq")
                    nc.vector.tensor_mul(sq[:], usl.bitcast(f32), usl.bitcast(f32))
                    ssq = npp.tile([128, 512], f32, name=f"ssq_{h}_{qb}", tag="ssq")
                    nc.tensor.matmul(ssq[:], ones_a[:], sq[:],
                                     start=True, stop=True)
                    sd = nsb.tile([128, 512], f32, name=f"sd_{h}_{qb}", tag="sd")
                    nc.scalar.activation(sd[:], ssq[:], Act.Sqrt,
                                         scale=1.0 / HD, bias=epsb[:])
                    rstd = nsb.tile([128, 512], f32, name=f"rstd_{h}_{qb}",
                                    tag="rstd")
                    nc.vector.reciprocal_approx_fast(rstd[:], sd[:])
                    nc.vector.tensor_mul(usl, usl.bitcast(f32), rstd[:])

        # ---------------- phase Y: output projection (partial) ----------------
        with tc.tile_pool(name="yw", bufs=1) as yw, \
             tc.tile_pool(name="yp", bufs=4, space="PSUM") as yp, \
             tc.tile_pool(name="ys", bufs=4) as ys:
            wo = [yw.tile([128, S], f32r, name=f"wo{h}", tag=f"wo{h}")
                  for h in range(HPC)]
            for h in range(HPC):
                nc.sync.dma_start(out=wo[h][:],
                                  in_=io["wo_t"][h * 128:(h + 1) * 128, :])
            for oc in range(NKC):
                for qc in range(NB):
                    py = yp.tile([128, 512], f32, name=f"py_{oc}_{qc}", tag="py")
                    for h in range(HPC):
                        nc.tensor.matmul(
                            py[:],
                            wo[h][:, oc * 128:(oc + 1) * 128],
                            U[h][:, qc * 512:(qc + 1) * 512],
                            start=(h == 0), stop=(h == HPC - 1))
                    yst = ys.tile([128, 512], f32, name=f"yst_{oc}_{qc}", tag="yst")
                    if (oc + qc) % 2 == 0:
                        nc.vector.tensor_copy(yst[:], py[:])
                    else:
                        nc.scalar.copy(yst[:], py[:])
                    nc.sync.dma_start(
                        out=y_t[oc * 128:(oc + 1) * 128, qc * 512:(qc + 1) * 512],
                        in_=yst[:])

    nc.compile()
    return nc


def _host_prep(q, k, v, Wq, Wk, Wv, Wo, lambda_q1, lambda_k1, lambda_q2,
               lambda_k2, gnorm_w, cos_emb, sin_emb):
    import ml_dtypes

    f32 = np.float32
    q = np.asarray(q, f32); k = np.asarray(k, f32); v = np.asarray(v, f32)
    Wq = np.asarray(Wq, f32); Wk = np.asarray(Wk, f32)
    Wv = np.asarray(Wv, f32); Wo = np.asarray(Wo, f32)
    gnorm_w = np.asarray(gnorm_w, f32)
    cos_emb = np.asarray(cos_emb, f32); sin_emb = np.asarray(sin_emb, f32)

    lam1 = np.exp(np.sum(np.asarray(lambda_q1, f32) * np.asarray(lambda_k1, f32),
                         dtype=f32))
    lam2 = np.exp(np.sum(np.asarray(lambda_q2, f32) * np.asarray(lambda_k2, f32),
                         dtype=f32))
    lam = np.float32(lam1 - lam2 + LAMBDA_INIT)

    # per-batch transposed activations
    xt = {}
    for b in range(B):
        xt[("q", b)] = np.ascontiguousarray(q[b].T)
        xt[("k", b)] = np.ascontiguousarray(k[b].T)
        xt[("v", b)] = np.ascontiguousarray(v[b].T)

    # shared constant tensors
    base_c = cos_emb[:S, :QKD]          # [S, 64]
    base_s = sin_emb[:S, :QKD]
    crep = np.ascontiguousarray(np.tile(base_c.T, (2, 1)))   # [128, S]
    srep = np.ascontiguousarray(np.tile(base_s.T, (2, 1)))
    pmat = np.zeros((128, 128), f32)
    for blk in range(2):
        o = blk * 64
        for i in range(QKD // 2):
            pmat[o + 2 * i, o + 2 * i + 1] = 1.0     # lhsT[2i, 2i+1]
            pmat[o + 2 * i + 1, o + 2 * i] = -1.0    # lhsT[2i+1, 2i]
    negi = (np.eye(128, dtype=f32) * np.float32(-1e9)).astype(ml_dtypes.bfloat16)
    utri = (np.tril(np.ones((128, 128), f32), -1)).astype(ml_dtypes.bfloat16)
    # utri[p, n] = 1 if p > n (key index > query index within the 128 block)
    ones_a = np.ones((128, 128), f32)
    neglam = np.full((128, 1), -lam, f32)

    per_core = []
    for c in range(NCORES):
        b, grp = c // GRPS, c % GRPS
        heads = [HPC * grp + j for j in range(HPC)]
        # wq/wk columns: tile t = 2*g + hp; within tile: head 2*hp+a at
        # cols [64*a, 64*a+64), original feature order (interleaved pairs)
        cols = []
        for t in range(4):
            g, hp = t // 2, t % 2
            for a2 in range(2):
                hg = heads[2 * hp + a2]
                cols.extend(hg * HD + g * QKD + d for d in range(QKD))
        cols = np.asarray(cols)
        vrows = np.asarray([h * HD + d for h in heads for d in range(HD)])
        wq_t = np.ascontiguousarray(Wq[cols, :].T)
        wk_t = np.ascontiguousarray(Wk[cols, :].T)
        wv_t = np.ascontiguousarray(Wv[vrows, :].T)
        gtile = np.tile(gnorm_w, HPC)                       # [512]
        wo_t = np.ascontiguousarray(
            ((1.0 - LAMBDA_INIT) * Wo[:, vrows] * gtile[None, :]).T)
        per_core.append({
            "xq_t": xt[("q", b)], "xk_t": xt[("k", b)], "xv_t": xt[("v", b)],
            "wq_t": wq_t, "wk_t": wk_t, "wv_t": wv_t, "wo_t": wo_t,
            "crep": crep, "srep": srep, "pmat": pmat,
            "negi": negi, "utri": utri, "ones_a": ones_a, "neglam": neglam,
        })
    return per_core


def _install_ntff_hook():
    """antenv.axon_hooks is absent in this image; synthesize it so
    run_bass_kernel_spmd(trace=True) can capture NTFF profiles."""
    import sys as _sys
    import types

    if "antenv.axon_hooks" in _sys.modules:
        return
    import antenv
    mod = types.ModuleType("antenv.axon_hooks")
    state = {"hook": None}
    mod.set_axon_ntff_profile_hook = lambda h: state.__setitem__("hook", h)
    mod.get_axon_ntff_profile_hook = lambda: state["hook"]
    _sys.modules["antenv.axon_hooks"] = mod
    antenv.axon_hooks = mod
    try:
        from trn_agent_boot.trn_boot import _ntff_profile_via_ctypes
        state["hook"] = _ntff_profile_via_ctypes("/opt/axon/libaxon_pjrt.so")
    except Exception as e:  # degrade: trace skipped, run still works
        print("ntff hook install failed:", e)


def kernel(q, k, v, Wq, Wk, Wv, Wo, lambda_q1, lambda_k1, lambda_q2,
           lambda_k2, gnorm_w, cos_emb, sin_emb, mask, _trace=False):
    if _trace:
        _install_ntff_hook()
    global _PROGRAM
    if _PROGRAM is None:
        _PROGRAM = _build_program()
    nc = _PROGRAM

    in_maps = _host_prep(q, k, v, Wq, Wk, Wv, Wo, lambda_q1, lambda_k1,
                         lambda_q2, lambda_k2, gnorm_w, cos_emb, sin_emb)

    from concourse.bass_utils import run_bass_kernel_spmd
    res = run_bass_kernel_spmd(nc, in_maps, core_ids=list(range(NCORES)),
                               trace=_trace)
    kernel.last_result = res

    y = np.zeros((B, S, HID), np.float32)
    for c in range(NCORES):
        y[c // GRPS] += res.results[c]["y_t"].T
    return y



# revision 5
# speedup vs baseline: 1.4087x; 1.4087x over previous
"""Multi-head differential attention on 8 Trainium2 NeuronCores.

Sharding: data-parallel over batch (B=2) x tensor-parallel over heads
(16 heads -> 4 per core). Core c handles batch c//4 and heads
4*(c%4) .. 4*(c%4)+3. Each core computes its heads' attention output and a
partial output projection; the host sums the 4 partials per batch.

v2: bf16 operands everywhere (matmul rate is identical to f32r but DVE runs
2-4x faster and DMA halves), rope fused into the projection phase, causal
diagonal masking moved from PE matmuls to elementwise DVE multiplies, RMS
partition-reduction moved to the idle GpSimd engine, and the output
projection interleaved per 512-query block so the PE stream never drains.
"""

import math
import os
import sys

sys.path.insert(0, "/opt/trn_rl_repo")

import numpy as np

B, S, HID, NH = 2, 2048, 2048, 16
HD = HID // NH          # 128
QKD = HD // 2           # 64
NCORES = 8
GRPS = NCORES // B      # head groups per batch
HPC = NH // GRPS        # heads per core = 4
LAYER_ID = 1
LAMBDA_INIT = 0.8 - 0.6 * math.exp(-0.3 * LAYER_ID)
EPS = 1e-6

NB = S // 512           # 4 seq blocks of 512
NKC = S // 128          # 16 key chunks of 128

_PROGRAM = None         # compiled bass program, reused across calls


def _build_program():
    import concourse.bass as bass
    import concourse.tile as tile
    from concourse import bacc, bass_isa, mybir

    f32 = mybir.dt.float32
    bf16 = mybir.dt.bfloat16
    Alu = mybir.AluOpType
    Act = mybir.ActivationFunctionType

    nc = bacc.Bacc(None, target_bir_lowering=False, debug=False)

    def din(name, shape, dt=bf16):
        return nc.dram_tensor(name, shape, dt, kind="ExternalInput").ap()

    io = {
        "xq_t": din("xq_t", [HID, S]),
        "xk_t": din("xk_t", [HID, S]),
        "xv_t": din("xv_t", [HID, S]),
        "wq_t": din("wq_t", [HID, 512]),
        "wk_t": din("wk_t", [HID, 512]),
        "wv_t": din("wv_t", [HID, 512]),
        "wo_t": din("wo_t", [512, HID]),
        "crep": din("crep", [128, S]),
        "srep": din("srep", [128, S]),
        "pmat": din("pmat", [128, 128]),
        "ones_a": din("ones_a", [128, 128]),
        "trimask": din("trimask", [128, 128]),
        "neglam": din("neglam", [128, 1], f32),
    }
    y_t = nc.dram_tensor("y_t", [HID, S], f32, kind="ExternalOutput").ap()

    from contextlib import ExitStack

    with tile.TileContext(nc) as tc, ExitStack() as ctx:
        persist = ctx.enter_context(tc.tile_pool(name="persist", bufs=1))
        constp = ctx.enter_context(tc.tile_pool(name="constp", bufs=1))

        # constants
        crep = constp.tile([128, S], bf16, name="crep_sb", tag="crep")
        srep = constp.tile([128, S], bf16, name="srep_sb", tag="srep")
        pmat = constp.tile([128, 128], bf16, name="pmat_sb", tag="pmat")
        ones_a = constp.tile([128, 128], bf16, name="ones_a_sb", tag="ones_a")
        trimask = constp.tile([128, 128], bf16, name="trimask_sb", tag="trimask")
        neglam = constp.tile([128, 1], f32, name="neglam_sb", tag="neglam")
        epsb = constp.tile([128, 1], f32, name="epsb", tag="epsb")
        nc.vector.memset(epsb[:], EPS)
        for t, key in ((crep, "crep"), (srep, "srep"), (pmat, "pmat"),
                       (ones_a, "ones_a"), (trimask, "trimask"),
                       (neglam, "neglam")):
            nc.sync.dma_start(out=t[:], in_=io[key][:])

        # persistent tensors: Q^T/K^T per (map g, head-pair hp): [128, S]
        #   tile t = 2*g + hp; partitions [64*a, 64*a+64) hold head 2*hp+a.
        QT = [persist.tile([128, S], bf16, name=f"qt{t}", tag=f"qt{t}")
              for t in range(4)]
        KT = [persist.tile([128, S], bf16, name=f"kt{t}", tag=f"kt{t}")
              for t in range(4)]
        # V natural layout per 128-seq chunk: [128 seq, 4 heads * 128 feat]
        VH = [persist.tile([128, 512], bf16, name=f"vh{s}", tag=f"vh{s}")
              for s in range(NKC)]
        # combined attention output (post RMS): [feat, seq] per head
        U = [persist.tile([128, S], bf16, name=f"u{h}", tag=f"u{h}")
             for h in range(HPC)]
        wo = [persist.tile([128, S], bf16, name=f"wo{h}", tag=f"wo{h}")
              for h in range(HPC)]

        # ---------------- phase P: q/k/v projections + fused rope ----------
        with tc.tile_pool(name="wp", bufs=1) as wp, \
             tc.tile_pool(name="xp", bufs=2) as xp, \
             tc.tile_pool(name="rsb", bufs=2) as rsb, \
             tc.tile_pool(name="pp", bufs=1, space="PSUM") as pp, \
             tc.tile_pool(name="pxp", bufs=2, space="PSUM") as pxp:
            wt = {}
            wnames = ("wq_t", "wk_t", "wv_t")

            def load_weights(mi):
                for kc in range(NKC):
                    w_ = wp.tile([128, 512], bf16, name=f"w{mi}_{kc}",
                                 tag=f"w{mi}_{kc}")
                    nc.sync.dma_start(
                        out=w_[:], in_=io[wnames[mi]][kc * 128:(kc + 1) * 128, :])
                    wt[(mi, kc)] = w_

            load_weights(0)
            for h in range(HPC):
                nc.sync.dma_start(out=wo[h][:],
                                  in_=io["wo_t"][h * 128:(h + 1) * 128, :])

            pending = []

            def flush_rope():
                T, t, n, raw = pending.pop()
                px = pxp.tile([128, 512], f32, name=f"px_{T[t].name}_{n}",
                              tag="px")
                nc.tensor.matmul(px[:], pmat[:], raw[:], start=True, stop=True)
                pxb = rsb.tile([128, 512], bf16, name=f"pxb_{T[t].name}_{n}",
                               tag="pxb")
                nc.scalar.copy(pxb[:], px[:])
                cs = slice(n * 512, (n + 1) * 512)
                tmp = rsb.tile([128, 512], bf16, name=f"tmp_{T[t].name}_{n}",
                               tag="tmp")
                nc.vector.tensor_mul(tmp[:], pxb[:], srep[:, cs])
                aa = rsb.tile([128, 512], bf16, name=f"aa_{T[t].name}_{n}",
                              tag="aa")
                nc.vector.tensor_mul(aa[:], raw[:], crep[:, cs])
                nc.vector.tensor_add(T[t][:, cs], aa[:], tmp[:])

            for mi in range(3):
                xin = io[("xq_t", "xk_t", "xv_t")[mi]]
                for n in range(NB):
                    xt = []
                    for kc in range(NKC):
                        xck = xp.tile([128, 512], bf16, name=f"x_{mi}_{n}_{kc}",
                                      tag=f"x{kc}")
                        nc.sync.dma_start(
                            out=xck[:],
                            in_=xin[kc * 128:(kc + 1) * 128,
                                    n * 512:(n + 1) * 512])
                        xt.append(xck)
                    if n == 0 and mi < 2:
                        load_weights(mi + 1)   # prefetch next mode's weights
                    for t in range(4):
                        ps = pp.tile([128, 512], f32, name=f"pp{t}_{mi}_{n}",
                                     tag=f"pp{t}")
                        for kc in range(NKC):
                            if mi < 2:
                                nc.tensor.matmul(ps[:],
                                                 wt[(mi, kc)][:, t * 128:(t + 1) * 128],
                                                 xt[kc][:],
                                                 start=(kc == 0), stop=(kc == 15))
                            else:
                                nc.tensor.matmul(ps[:],
                                                 xt[kc][:, t * 128:(t + 1) * 128],
                                                 wt[(mi, kc)][:],
                                                 start=(kc == 0), stop=(kc == 15))
                            if kc == 2 and pending:
                                flush_rope()
                        if mi == 2:
                            nc.scalar.copy(VH[n * 4 + t][:], ps[:])
                        else:
                            raw = rsb.tile([128, 512], bf16,
                                           name=f"raw_{mi}_{n}_{t}", tag="raw")
                            nc.scalar.copy(raw[:], ps[:])
                            pending.append((QT if mi == 0 else KT, t, n, raw))
            while pending:
                flush_rope()

        # ---------------- phase A: attention + rms + output proj ----------
        with tc.tile_pool(name="sp", bufs=2, space="PSUM") as sp, \
             tc.tile_pool(name="pvp", bufs=1, space="PSUM") as pvp, \
             tc.tile_pool(name="smp", bufs=1, space="PSUM") as smp, \
             tc.tile_pool(name="ep", bufs=3) as ep, \
             tc.tile_pool(name="cb", bufs=2) as cb, \
             tc.tile_pool(name="ys", bufs=2) as ys:
            for qb in range(NB):
                for h in range(HPC):
                    hp, a = h // 2, h % 2
                    poff = 64 * a
                    pv = pvp.tile([128, 1024], f32, name=f"pv_{h}_{qb}",
                                  tag="pv")
                    sm = smp.tile([128, 1024], f32, name=f"sm_{h}_{qb}",
                                  tag="sm")
                    nkc = 4 * qb + 4
                    for kc in range(nkc):
                        j = kc - 4 * qb
                        qoff = max(j, 0) * 128
                        ps = sp.tile([128, 1024], f32, name=f"s_{h}_{qb}_{kc}",
                                     tag="s")
                        for g in (0, 1):
                            tq = 2 * g + hp
                            nc.tensor.matmul(
                                ps[:, g * 512 + qoff:(g + 1) * 512],
                                KT[tq][poff:poff + 64,
                                       kc * 128:(kc + 1) * 128],
                                QT[tq][poff:poff + 64,
                                       qb * 512 + qoff:(qb + 1) * 512],
                                start=True, stop=True)
                        E = ep.tile([128, 1024], bf16, name=f"e_{h}_{qb}_{kc}",
                                    tag="e")
                        if qoff == 0:
                            nc.scalar.activation(E[:], ps[:], Act.Exp,
                                                 scale=0.125)
                        else:
                            for g in (0, 1):
                                nc.scalar.activation(
                                    E[:, g * 512 + qoff:(g + 1) * 512],
                                    ps[:, g * 512 + qoff:(g + 1) * 512],
                                    Act.Exp, scale=0.125)
                        if j >= 0:
                            for g in (0, 1):
                                sl = E[:, g * 512 + qoff:g * 512 + qoff + 128]
                                nc.vector.tensor_mul(sl, sl, trimask[:])
                        first, last = (kc == 0), (kc == nkc - 1)
                        for g in (0, 1):
                            sl = slice(g * 512 + qoff, (g + 1) * 512)
                            nc.tensor.matmul(
                                pv[:, sl],
                                VH[kc][:, h * 128:(h + 1) * 128],
                                E[:, sl], start=first, stop=last)
                            nc.tensor.matmul(
                                sm[:, sl], ones_a[:], E[:, sl],
                                start=first, stop=last)
                    # combine: U = pv1/sm1 - lam * pv2/sm2, then RMS norm
                    rb = cb.tile([128, 1024], f32, name=f"rb_{h}_{qb}",
                                 tag="rb")
                    nc.vector.reciprocal_approx_fast(rb[:], sm[:])
                    tt = cb.tile([128, 1024], f32, name=f"tt_{h}_{qb}",
                                 tag="tt")
                    nc.vector.tensor_mul(tt[:], pv[:], rb[:])
                    dst = U[h][:, qb * 512:(qb + 1) * 512]
                    nc.vector.scalar_tensor_tensor(
                        dst, tt[:, 512:1024], neglam[:], tt[:, 0:512],
                        op0=Alu.mult, op1=Alu.add)
                    sq = cb.tile([128, 512], bf16, name=f"sq_{h}_{qb}",
                                 tag="sq")
                    nc.vector.tensor_mul(sq[:], dst, dst)
                    ssq = cb.tile([128, 512], f32, name=f"ssq_{h}_{qb}",
                                  tag="ssq")
                    nc.gpsimd.partition_all_reduce(ssq[:], sq[:], 128,
                                                   bass_isa.ReduceOp.add)
                    sd = cb.tile([128, 512], f32, name=f"sd_{h}_{qb}",
                                 tag="sd")
                    nc.scalar.activation(sd[:], ssq[:], Act.Sqrt,
                                         scale=1.0 / HD, bias=epsb[:])
                    rstd = cb.tile([128, 512], f32, name=f"rstd_{h}_{qb}",
                                   tag="rstd")
                    nc.vector.reciprocal_approx_fast(rstd[:], sd[:])
                    rstdb = cb.tile([128, 512], bf16, name=f"rstdb_{h}_{qb}",
                                    tag="rstdb")
                    nc.scalar.copy(rstdb[:], rstd[:])
                    nc.vector.tensor_mul(dst, dst, rstdb[:])
                # output projection for this query block (fills PE pipeline
                # while the next block's first exp runs)
                for pr in range(8):
                    psy = sp.tile([128, 1024], f32, name=f"py_{qb}_{pr}",
                                  tag="s")
                    for i in (0, 1):
                        oc = 2 * pr + i
                        for h2 in range(HPC):
                            nc.tensor.matmul(
                                psy[:, i * 512:(i + 1) * 512],
                                wo[h2][:, oc * 128:(oc + 1) * 128],
                                U[h2][:, qb * 512:(qb + 1) * 512],
                                start=(h2 == 0), stop=(h2 == HPC - 1))
                    yst = ys.tile([128, 1024], f32, name=f"yst_{qb}_{pr}",
                                  tag="yst")
                    if pr % 2 == 0:
                        nc.vector.tensor_copy(yst[:], psy[:])
                    else:
                        nc.scalar.copy(yst[:], psy[:])
                    for i in (0, 1):
                        oc = 2 * pr + i
                        nc.sync.dma_start(
                            out=y_t[oc * 128:(oc + 1) * 128,
                                    qb * 512:(qb + 1) * 512],
                            in_=yst[:, i * 512:(i + 1) * 512])

    nc.compile()
    return nc


def _host_prep(q, k, v, Wq, Wk, Wv, Wo, lambda_q1, lambda_k1, lambda_q2,
               lambda_k2, gnorm_w, cos_emb, sin_emb):
    import ml_dtypes

    f32 = np.float32
    bf16 = ml_dtypes.bfloat16
    q = np.asarray(q, f32); k = np.asarray(k, f32); v = np.asarray(v, f32)
    Wq = np.asarray(Wq, f32); Wk = np.asarray(Wk, f32)
    Wv = np.asarray(Wv, f32); Wo = np.asarray(Wo, f32)
    gnorm_w = np.asarray(gnorm_w, f32)
    cos_emb = np.asarray(cos_emb, f32); sin_emb = np.asarray(sin_emb, f32)

    lam1 = np.exp(np.sum(np.asarray(lambda_q1, f32) * np.asarray(lambda_k1, f32),
                         dtype=f32))
    lam2 = np.exp(np.sum(np.asarray(lambda_q2, f32) * np.asarray(lambda_k2, f32),
                         dtype=f32))
    lam = np.float32(lam1 - lam2 + LAMBDA_INIT)

    # per-batch transposed activations (bf16)
    xt = {}
    for b in range(B):
        xt[("q", b)] = np.ascontiguousarray(q[b].T).astype(bf16)
        xt[("k", b)] = np.ascontiguousarray(k[b].T).astype(bf16)
        xt[("v", b)] = np.ascontiguousarray(v[b].T).astype(bf16)

    # shared constant tensors
    base_c = cos_emb[:S, :QKD]          # [S, 64]
    base_s = sin_emb[:S, :QKD]
    crep = np.ascontiguousarray(np.tile(base_c.T, (2, 1))).astype(bf16)
    srep = np.ascontiguousarray(np.tile(base_s.T, (2, 1))).astype(bf16)
    pmat = np.zeros((128, 128), f32)
    for blk in range(2):
        o = blk * 64
        for i in range(QKD // 2):
            pmat[o + 2 * i, o + 2 * i + 1] = 1.0     # lhsT[2i, 2i+1]
            pmat[o + 2 * i + 1, o + 2 * i] = -1.0    # lhsT[2i+1, 2i]
    pmat = pmat.astype(bf16)
    ones_a = np.ones((128, 128), f32).astype(bf16)
    # trimask[p, n] = 1 if key-in-chunk p <= query-in-block n (valid)
    trimask = np.triu(np.ones((128, 128), f32), 0).astype(bf16)
    neglam = np.full((128, 1), -lam, f32)

    per_core = []
    for c in range(NCORES):
        b, grp = c // GRPS, c % GRPS
        heads = [HPC * grp + j for j in range(HPC)]
        # wq/wk columns: tile t = 2*g + hp; within tile: head 2*hp+a at
        # cols [64*a, 64*a+64), original feature order (interleaved pairs)
        cols = []
        for t in range(4):
            g, hp = t // 2, t % 2
            for a2 in range(2):
                hg = heads[2 * hp + a2]
                cols.extend(hg * HD + g * QKD + d for d in range(QKD))
        cols = np.asarray(cols)
        vrows = np.asarray([h * HD + d for h in heads for d in range(HD)])
        wq_t = np.ascontiguousarray(Wq[cols, :].T).astype(bf16)
        wk_t = np.ascontiguousarray(Wk[cols, :].T).astype(bf16)
        wv_t = np.ascontiguousarray(Wv[vrows, :].T).astype(bf16)
        gtile = np.tile(gnorm_w, HPC)                       # [512]
        wo_t = np.ascontiguousarray(
            ((1.0 - LAMBDA_INIT) * Wo[:, vrows] * gtile[None, :]).T).astype(bf16)
        per_core.append({
            "xq_t": xt[("q", b)], "xk_t": xt[("k", b)], "xv_t": xt[("v", b)],
            "wq_t": wq_t, "wk_t": wk_t, "wv_t": wv_t, "wo_t": wo_t,
            "crep": crep, "srep": srep, "pmat": pmat,
            "ones_a": ones_a, "trimask": trimask, "neglam": neglam,
        })
    return per_core


def _install_ntff_hook():
    """antenv.axon_hooks is absent in this image; synthesize it so
    run_bass_kernel_spmd(trace=True) can capture NTFF profiles."""
    import sys as _sys
    import types

    if "antenv.axon_hooks" in _sys.modules:
        return
    import antenv
    mod = types.ModuleType("antenv.axon_hooks")
    state = {"hook": None}
    mod.set_axon_ntff_profile_hook = lambda h: state.__setitem__("hook", h)
    mod.get_axon_ntff_profile_hook = lambda: state["hook"]
    _sys.modules["antenv.axon_hooks"] = mod
    antenv.axon_hooks = mod
    try:
        from trn_agent_boot.trn_boot import _ntff_profile_via_ctypes
        state["hook"] = _ntff_profile_via_ctypes("/opt/axon/libaxon_pjrt.so")
    except Exception as e:  # degrade: trace skipped, run still works
        print("ntff hook install failed:", e)


def kernel(q, k, v, Wq, Wk, Wv, Wo, lambda_q1, lambda_k1, lambda_q2,
           lambda_k2, gnorm_w, cos_emb, sin_emb, mask, _trace=False):
    if _trace:
        _install_ntff_hook()
    global _PROGRAM
    if _PROGRAM is None:
        _PROGRAM = _build_program()
    nc = _PROGRAM

    in_maps = _host_prep(q, k, v, Wq, Wk, Wv, Wo, lambda_q1, lambda_k1,
                         lambda_q2, lambda_k2, gnorm_w, cos_emb, sin_emb)

    from concourse.bass_utils import run_bass_kernel_spmd
    res = run_bass_kernel_spmd(nc, in_maps, core_ids=list(range(NCORES)),
                               trace=_trace)
    kernel.last_result = res

    y = np.zeros((B, S, HID), np.float32)
    for c in range(NCORES):
        y[c // GRPS] += res.results[c]["y_t"].T
    return y


# revision 10
# speedup vs baseline: 1.5154x; 1.0757x over previous
"""Multi-head differential attention on 8 Trainium2 NeuronCores.

Sharding: data-parallel over batch (B=2) x tensor-parallel over heads
(16 heads -> 4 per core). Core c handles batch c//4 and heads
4*(c%4) .. 4*(c%4)+3. Each core computes its heads' attention output and a
partial output projection; the host sums the 4 partials per batch.

v2: bf16 operands everywhere (matmul rate is identical to f32r but DVE runs
2-4x faster and DMA halves), rope fused into the projection phase, causal
diagonal masking moved from PE matmuls to elementwise DVE multiplies, RMS
partition-reduction moved to the idle GpSimd engine, and the output
projection interleaved per 512-query block so the PE stream never drains.
"""

import math
import os
import sys

sys.path.insert(0, "/opt/trn_rl_repo")

import numpy as np

B, S, HID, NH = 2, 2048, 2048, 16
HD = HID // NH          # 128
QKD = HD // 2           # 64
NCORES = 8
GRPS = NCORES // B      # head groups per batch
HPC = NH // GRPS        # heads per core = 4
LAYER_ID = 1
LAMBDA_INIT = 0.8 - 0.6 * math.exp(-0.3 * LAYER_ID)
EPS = 1e-6

NB = S // 512           # 4 seq blocks of 512
NKC = S // 128          # 16 key chunks of 128

_PROGRAM = None         # compiled bass program, reused across calls


def _build_program():
    import concourse.bass as bass
    import concourse.tile as tile
    from concourse import bacc, bass_isa, mybir

    f32 = mybir.dt.float32
    bf16 = mybir.dt.bfloat16
    Alu = mybir.AluOpType
    Act = mybir.ActivationFunctionType

    nc = bacc.Bacc(None, target_bir_lowering=False, debug=False)

    def din(name, shape, dt=bf16):
        return nc.dram_tensor(name, shape, dt, kind="ExternalInput").ap()

    io = {
        "xq_t": din("xq_t", [HID, S]),
        "xk_t": din("xk_t", [HID, S]),
        "xv_t": din("xv_t", [HID, S]),
        "wq_t": din("wq_t", [HID, 512]),
        "wk_t": din("wk_t", [HID, 512]),
        "wv_t": din("wv_t", [HID, 512]),
        "wo_t": din("wo_t", [512, HID]),
        "crep": din("crep", [128, S]),
        "srep": din("srep", [128, S]),
        "pmat": din("pmat", [128, 128]),
        "ones_a": din("ones_a", [128, 128]),
        "trimask": din("trimask", [128, 128]),
        "neglam": din("neglam", [128, 1], f32),
    }
    y_t = nc.dram_tensor("y_t", [HID, S], f32, kind="ExternalOutput").ap()

    from contextlib import ExitStack

    with tile.TileContext(nc) as tc, ExitStack() as ctx:
        persist = ctx.enter_context(tc.tile_pool(name="persist", bufs=1))
        constp = ctx.enter_context(tc.tile_pool(name="constp", bufs=1))

        # constants
        crep = constp.tile([128, S], bf16, name="crep_sb", tag="crep")
        srep = constp.tile([128, S], bf16, name="srep_sb", tag="srep")
        pmat = constp.tile([128, 128], bf16, name="pmat_sb", tag="pmat")
        ones_a = constp.tile([128, 128], bf16, name="ones_a_sb", tag="ones_a")
        trimask = constp.tile([128, 128], bf16, name="trimask_sb", tag="trimask")
        neglam = constp.tile([128, 1], f32, name="neglam_sb", tag="neglam")
        epsb = constp.tile([128, 1], f32, name="epsb", tag="epsb")
        nc.vector.memset(epsb[:], EPS)
        for t, key in ((crep, "crep"), (srep, "srep"), (pmat, "pmat"),
                       (ones_a, "ones_a"), (trimask, "trimask"),
                       (neglam, "neglam")):
            nc.sync.dma_start(out=t[:], in_=io[key][:])

        # persistent tensors: Q^T/K^T per (map g, head-pair hp): [128, S]
        #   tile t = 2*g + hp; partitions [64*a, 64*a+64) hold head 2*hp+a.
        QT = [persist.tile([128, S], bf16, name=f"qt{t}", tag=f"qt{t}")
              for t in range(4)]
        KT = [persist.tile([128, S], bf16, name=f"kt{t}", tag=f"kt{t}")
              for t in range(4)]
        # V natural layout per 128-seq chunk: [128 seq, 4 heads * 128 feat]
        VH = [persist.tile([128, 512], bf16, name=f"vh{s}", tag=f"vh{s}")
              for s in range(NKC)]
        # combined attention output (post RMS): [feat, seq] per head
        U = [persist.tile([128, S], bf16, name=f"u{h}", tag=f"u{h}")
             for h in range(HPC)]
        wo = [persist.tile([128, S], bf16, name=f"wo{h}", tag=f"wo{h}")
              for h in range(HPC)]

        # ---------------- phase P: q/k/v projections + fused rope ----------
        with tc.tile_pool(name="wp", bufs=1) as wp, \
             tc.tile_pool(name="xp", bufs=2) as xp, \
             tc.tile_pool(name="rsb", bufs=2) as rsb, \
             tc.tile_pool(name="pp", bufs=1, space="PSUM") as pp, \
             tc.tile_pool(name="pxp", bufs=2, space="PSUM") as pxp:
            wt = {}
            wnames = ("wq_t", "wk_t", "wv_t")

            def load_weights(mi):
                for kc in range(NKC):
                    w_ = wp.tile([128, 512], bf16, name=f"w{mi}_{kc}",
                                 tag=f"w{mi}_{kc}")
                    nc.sync.dma_start(
                        out=w_[:], in_=io[wnames[mi]][kc * 128:(kc + 1) * 128, :])
                    wt[(mi, kc)] = w_

            load_weights(0)
            for h in range(HPC):
                nc.sync.dma_start(out=wo[h][:],
                                  in_=io["wo_t"][h * 128:(h + 1) * 128, :])

            pending = []

            def flush_rope():
                T, t, n, raw = pending.pop()
                px = pxp.tile([128, 512], f32, name=f"px_{T[t].name}_{n}",
                              tag="px")
                nc.tensor.matmul(px[:], pmat[:], raw[:], start=True, stop=True)
                pxb = rsb.tile([128, 512], bf16, name=f"pxb_{T[t].name}_{n}",
                               tag="pxb")
                nc.scalar.copy(pxb[:], px[:])
                cs = slice(n * 512, (n + 1) * 512)
                tmp = rsb.tile([128, 512], bf16, name=f"tmp_{T[t].name}_{n}",
                               tag="tmp")
                nc.vector.tensor_mul(tmp[:], pxb[:], srep[:, cs])
                aa = rsb.tile([128, 512], bf16, name=f"aa_{T[t].name}_{n}",
                              tag="aa")
                nc.vector.tensor_mul(aa[:], raw[:], crep[:, cs])
                nc.vector.tensor_add(T[t][:, cs], aa[:], tmp[:])

            for mi in range(3):
                xin = io[("xq_t", "xk_t", "xv_t")[mi]]
                for n in range(NB):
                    xt = []
                    for kc in range(NKC):
                        xck = xp.tile([128, 512], bf16, name=f"x_{mi}_{n}_{kc}",
                                      tag=f"x{kc}")
                        nc.sync.dma_start(
                            out=xck[:],
                            in_=xin[kc * 128:(kc + 1) * 128,
                                    n * 512:(n + 1) * 512])
                        xt.append(xck)
                    if n == 0 and mi < 2:
                        load_weights(mi + 1)   # prefetch next mode's weights
                    for t in range(4):
                        ps = pp.tile([128, 512], f32, name=f"pp{t}_{mi}_{n}",
                                     tag=f"pp{t}")
                        for kc in range(NKC):
                            if mi < 2:
                                nc.tensor.matmul(ps[:],
                                                 wt[(mi, kc)][:, t * 128:(t + 1) * 128],
                                                 xt[kc][:],
                                                 start=(kc == 0), stop=(kc == 15))
                            else:
                                nc.tensor.matmul(ps[:],
                                                 xt[kc][:, t * 128:(t + 1) * 128],
                                                 wt[(mi, kc)][:],
                                                 start=(kc == 0), stop=(kc == 15))
                            if kc == 2 and pending:
                                flush_rope()
                        if mi == 2:
                            nc.scalar.copy(VH[n * 4 + t][:], ps[:])
                        else:
                            raw = rsb.tile([128, 512], bf16,
                                           name=f"raw_{mi}_{n}_{t}", tag="raw")
                            nc.vector.tensor_copy(raw[:], ps[:])
                            pending.append((QT if mi == 0 else KT, t, n, raw))
            while pending:
                flush_rope()

        # ---------------- phase A: attention + rms + output proj ----------
        with tc.tile_pool(name="sp", bufs=2, space="PSUM") as sp, \
             tc.tile_pool(name="pvp", bufs=1, space="PSUM") as pvp, \
             tc.tile_pool(name="smp", bufs=1, space="PSUM") as smp, \
             tc.tile_pool(name="ep", bufs=3) as ep, \
             tc.tile_pool(name="cb", bufs=2) as cb, \
             tc.tile_pool(name="ys", bufs=2) as ys:
            pending_rms = []

            def flush_rms():
                # rms tail for a completed (h, qb): partition-sum of U^2 on
                # the PE, then rstd = exp(-0.5*ln(mean+eps)) on the scalar
                # engine (ln/exp share an activation table; sqrt does not).
                h_, qb_, sq_, dst_ = pending_rms.pop()
                ssq_t = sp.tile([128, 1024], f32, name=f"ssq_{h_}_{qb_}",
                                tag="s")
                nc.tensor.matmul(ssq_t[:, 0:512], ones_a[:], sq_[:],
                                 start=True, stop=True)
                lnm = cb.tile([128, 512], f32, name=f"lnm_{h_}_{qb_}",
                              tag="lnm")
                nc.scalar.activation(lnm[:], ssq_t[:, 0:512], Act.Ln,
                                     scale=1.0 / HD, bias=epsb[:])
                rstdb = cb.tile([128, 512], bf16, name=f"rstdb_{h_}_{qb_}",
                                tag="rstdb")
                nc.scalar.activation(rstdb[:], lnm[:], Act.Exp, scale=-0.5)
                nc.vector.tensor_mul(dst_, dst_, rstdb[:])

            for qb in range(NB):
                for h in range(HPC):
                    hp, a = h // 2, h % 2
                    poff = 64 * a
                    pv = pvp.tile([128, 1024], f32, name=f"pv_{h}_{qb}",
                                  tag="pv")
                    sm = smp.tile([128, 1024], f32, name=f"sm_{h}_{qb}",
                                  tag="sm")
                    nkc = 4 * qb + 4
                    for kc in range(nkc):
                        j = kc - 4 * qb
                        qoff = max(j, 0) * 128
                        ps = sp.tile([128, 1024], f32, name=f"s_{h}_{qb}_{kc}",
                                     tag="s")
                        if kc == 2 and pending_rms:
                            flush_rms()
                        for g in (0, 1):
                            tq = 2 * g + hp
                            nc.tensor.matmul(
                                ps[:, g * 512 + qoff:(g + 1) * 512],
                                KT[tq][poff:poff + 64,
                                       kc * 128:(kc + 1) * 128],
                                QT[tq][poff:poff + 64,
                                       qb * 512 + qoff:(qb + 1) * 512],
                                start=True, stop=True)
                        E = ep.tile([128, 1024], bf16, name=f"e_{h}_{qb}_{kc}",
                                    tag="e")
                        if qoff == 0:
                            nc.scalar.activation(E[:], ps[:], Act.Exp,
                                                 scale=0.125)
                        else:
                            for g in (0, 1):
                                nc.scalar.activation(
                                    E[:, g * 512 + qoff:(g + 1) * 512],
                                    ps[:, g * 512 + qoff:(g + 1) * 512],
                                    Act.Exp, scale=0.125)
                        if j >= 0:
                            for g in (0, 1):
                                sl = E[:, g * 512 + qoff:g * 512 + qoff + 128]
                                nc.vector.tensor_mul(sl, sl, trimask[:])
                        first, last = (kc == 0), (kc == nkc - 1)
                        for g in (0, 1):
                            sl = slice(g * 512 + qoff, (g + 1) * 512)
                            nc.tensor.matmul(
                                pv[:, sl],
                                VH[kc][:, h * 128:(h + 1) * 128],
                                E[:, sl], start=first, stop=last)
                            nc.tensor.matmul(
                                sm[:, sl], ones_a[:], E[:, sl],
                                start=first, stop=last)
                    # combine: U = pv1/sm1 - lam * pv2/sm2, then RMS norm
                    rb = cb.tile([128, 1024], f32, name=f"rb_{h}_{qb}",
                                 tag="rb")
                    nc.vector.reciprocal_approx_fast(rb[:], sm[:])
                    tt = cb.tile([128, 1024], f32, name=f"tt_{h}_{qb}",
                                 tag="tt")
                    nc.vector.tensor_mul(tt[:], pv[:], rb[:])
                    dst = U[h][:, qb * 512:(qb + 1) * 512]
                    nc.vector.scalar_tensor_tensor(
                        dst, tt[:, 512:1024], neglam[:], tt[:, 0:512],
                        op0=Alu.mult, op1=Alu.add)
                    sq = cb.tile([128, 512], bf16, name=f"sq_{h}_{qb}",
                                 tag="sq")
                    nc.vector.tensor_mul(sq[:], dst, dst)
                    pending_rms.append((h, qb, sq, dst))
                while pending_rms:
                    flush_rms()
                # output projection for this query block (fills PE pipeline
                # while the next block's first exp runs)
                for pr in range(8):
                    psy = sp.tile([128, 1024], f32, name=f"py_{qb}_{pr}",
                                  tag="s")
                    for i in (0, 1):
                        oc = 2 * pr + i
                        for h2 in range(HPC):
                            nc.tensor.matmul(
                                psy[:, i * 512:(i + 1) * 512],
                                wo[h2][:, oc * 128:(oc + 1) * 128],
                                U[h2][:, qb * 512:(qb + 1) * 512],
                                start=(h2 == 0), stop=(h2 == HPC - 1))
                    yst = ys.tile([128, 1024], f32, name=f"yst_{qb}_{pr}",
                                  tag="yst")
                    if pr % 2 == 0:
                        nc.vector.tensor_copy(yst[:], psy[:])
                    else:
                        nc.scalar.copy(yst[:], psy[:])
                    for i in (0, 1):
                        oc = 2 * pr + i
                        nc.sync.dma_start(
                            out=y_t[oc * 128:(oc + 1) * 128,
                                    qb * 512:(qb + 1) * 512],
                            in_=yst[:, i * 512:(i + 1) * 512])

    nc.compile()
    return nc


def _host_prep(q, k, v, Wq, Wk, Wv, Wo, lambda_q1, lambda_k1, lambda_q2,
               lambda_k2, gnorm_w, cos_emb, sin_emb):
    import ml_dtypes

    f32 = np.float32
    bf16 = ml_dtypes.bfloat16
    q = np.asarray(q, f32); k = np.asarray(k, f32); v = np.asarray(v, f32)
    Wq = np.asarray(Wq, f32); Wk = np.asarray(Wk, f32)
    Wv = np.asarray(Wv, f32); Wo = np.asarray(Wo, f32)
    gnorm_w = np.asarray(gnorm_w, f32)
    cos_emb = np.asarray(cos_emb, f32); sin_emb = np.asarray(sin_emb, f32)

    lam1 = np.exp(np.sum(np.asarray(lambda_q1, f32) * np.asarray(lambda_k1, f32),
                         dtype=f32))
    lam2 = np.exp(np.sum(np.asarray(lambda_q2, f32) * np.asarray(lambda_k2, f32),
                         dtype=f32))
    lam = np.float32(lam1 - lam2 + LAMBDA_INIT)

    # per-batch transposed activations (bf16)
    xt = {}
    for b in range(B):
        xt[("q", b)] = np.ascontiguousarray(q[b].T).astype(bf16)
        xt[("k", b)] = np.ascontiguousarray(k[b].T).astype(bf16)
        xt[("v", b)] = np.ascontiguousarray(v[b].T).astype(bf16)

    # shared constant tensors
    base_c = cos_emb[:S, :QKD]          # [S, 64]
    base_s = sin_emb[:S, :QKD]
    crep = np.ascontiguousarray(np.tile(base_c.T, (2, 1))).astype(bf16)
    srep = np.ascontiguousarray(np.tile(base_s.T, (2, 1))).astype(bf16)
    pmat = np.zeros((128, 128), f32)
    for blk in range(2):
        o = blk * 64
        for i in range(QKD // 2):
            pmat[o + 2 * i, o + 2 * i + 1] = 1.0     # lhsT[2i, 2i+1]
            pmat[o + 2 * i + 1, o + 2 * i] = -1.0    # lhsT[2i+1, 2i]
    pmat = pmat.astype(bf16)
    ones_a = np.ones((128, 128), f32).astype(bf16)
    # trimask[p, n] = 1 if key-in-chunk p <= query-in-block n (valid)
    trimask = np.triu(np.ones((128, 128), f32), 0).astype(bf16)
    neglam = np.full((128, 1), -lam, f32)

    per_core = []
    for c in range(NCORES):
        b, grp = c // GRPS, c % GRPS
        heads = [HPC * grp + j for j in range(HPC)]
        # wq/wk columns: tile t = 2*g + hp; within tile: head 2*hp+a at
        # cols [64*a, 64*a+64), original feature order (interleaved pairs)
        cols = []
        for t in range(4):
            g, hp = t // 2, t % 2
            for a2 in range(2):
                hg = heads[2 * hp + a2]
                cols.extend(hg * HD + g * QKD + d for d in range(QKD))
        cols = np.asarray(cols)
        vrows = np.asarray([h * HD + d for h in heads for d in range(HD)])
        wq_t = np.ascontiguousarray(Wq[cols, :].T).astype(bf16)
        wk_t = np.ascontiguousarray(Wk[cols, :].T).astype(bf16)
        wv_t = np.ascontiguousarray(Wv[vrows, :].T).astype(bf16)
        gtile = np.tile(gnorm_w, HPC)                       # [512]
        wo_t = np.ascontiguousarray(
            ((1.0 - LAMBDA_INIT) * Wo[:, vrows] * gtile[None, :]).T).astype(bf16)
        per_core.append({
            "xq_t": xt[("q", b)], "xk_t": xt[("k", b)], "xv_t": xt[("v", b)],
            "wq_t": wq_t, "wk_t": wk_t, "wv_t": wv_t, "wo_t": wo_t,
            "crep": crep, "srep": srep, "pmat": pmat,
            "ones_a": ones_a, "trimask": trimask, "neglam": neglam,
        })
    return per_core


def _install_ntff_hook():
    """antenv.axon_hooks is absent in this image; synthesize it so
    run_bass_kernel_spmd(trace=True) can capture NTFF profiles."""
    import sys as _sys
    import types

    if "antenv.axon_hooks" in _sys.modules:
        return
    import antenv
    mod = types.ModuleType("antenv.axon_hooks")
    state = {"hook": None}
    mod.set_axon_ntff_profile_hook = lambda h: state.__setitem__("hook", h)
    mod.get_axon_ntff_profile_hook = lambda: state["hook"]
    _sys.modules["antenv.axon_hooks"] = mod
    antenv.axon_hooks = mod
    try:
        from trn_agent_boot.trn_boot import _ntff_profile_via_ctypes
        state["hook"] = _ntff_profile_via_ctypes("/opt/axon/libaxon_pjrt.so")
    except Exception as e:  # degrade: trace skipped, run still works
        print("ntff hook install failed:", e)


def kernel(q, k, v, Wq, Wk, Wv, Wo, lambda_q1, lambda_k1, lambda_q2,
           lambda_k2, gnorm_w, cos_emb, sin_emb, mask, _trace=False):
    if _trace:
        _install_ntff_hook()
    global _PROGRAM
    if _PROGRAM is None:
        _PROGRAM = _build_program()
    nc = _PROGRAM

    in_maps = _host_prep(q, k, v, Wq, Wk, Wv, Wo, lambda_q1, lambda_k1,
                         lambda_q2, lambda_k2, gnorm_w, cos_emb, sin_emb)

    from concourse.bass_utils import run_bass_kernel_spmd
    res = run_bass_kernel_spmd(nc, in_maps, core_ids=list(range(NCORES)),
                               trace=_trace)
    kernel.last_result = res

    y = np.zeros((B, S, HID), np.float32)
    for c in range(NCORES):
        y[c // GRPS] += res.results[c]["y_t"].T
    return y
